# revision 18
# baseline (speedup 1.0000x reference)
"""Collaborative attention (nn_CollaborativeAttention) on 8 Trainium2 NeuronCores.

Reference math (B=2, S=2048, D=1024, H=16 heads, head mixing over full DKQ=1024):
    q = h @ Wq.T ; k = h @ Wk.T ; v = h @ Wv.T + bv
    scores[b,h,s,t] = sum_e q[b,s,e] * mixing[h,e] * k[b,t,e] / sqrt(64)
    probs = softmax_t(scores) ; ctx[b,s,:] = concat_h(probs @ v_head)

Sharding: core c handles batch b = c//4 and head group g = c%4 (4 heads each).
No cross-core communication; host slices inputs / concatenates outputs.

Device dataflow (per core, all matmuls f16 with fp32 PSUM accumulation):
    hT_aug [1152,2048]  host-transposed hidden (+ ones row for the v bias)
    kT[e,t]  = WkT.T @ hT          (stays transposed: e on partitions)
    v[t,dv]  = hT.T @ WvT_aug      (natural; padded to 128 cols: col 64 = ones
                                    row -> softmax denominator, cols 65.. = 0)
    per s-block of 512 queries:
      mq[e,s]   = (WqT.T @ hT_s) * mixing[h,e]        (per head)
      scoresT   = kT_chunk.T @ mq  -> exp(0.125*x) -> expT[t,s] (f16)
      ctxT_aug  = sum_t v_chunk.T @ expT   (PSUM accum over 16 t-chunks;
                 rows 0..63 = unnormalized ctxT, row 64 = denominator)
      PE-transpose ctxT_aug 128x128 blocks -> ctx[s,dv] ; multiply by
      reciprocal(denominator) ; DMA out.
"""

import math

import numpy as np
import ml_dtypes

B, S, D = 2, 2048, 1024
H, DV = 16, 1024
N_CORES = 8
HG = 4  # heads per core
DH = 64  # head dim
P = 128
EC = 8  # e-chunks (1024/128)
DC = 9  # d-chunks incl. bias row (1152/128)
NB = 512  # s-block width
SBLK = 4  # number of s blocks
TCH = 16  # t-chunks of 128
SCALE = 1.0 / math.sqrt(D / H)  # 0.125

_CACHE: dict = {}


def build_program():
    """Build the (SPMD, per-core) Bass program. Returns nc (compiled Bacc —
    Bacc.compile() runs the wait-legalization passes walrus needs: at most one
    semaphore wait per instruction, excess split into EventSemaphore chains)."""
    import concourse.bass as bass
    import concourse.mybir as mybir
    from concourse import bacc
    from concourse.tile import TileContext
    from concourse.masks import make_identity

    f32 = mybir.dt.float32
    f16 = mybir.dt.float16
    mult = mybir.AluOpType.mult
    Exp = mybir.ActivationFunctionType.Exp
    Copy = mybir.ActivationFunctionType.Copy

    nc = bacc.Bacc("TRN2", target_bir_lowering=False, debug=True)
    ht = nc.dram_tensor("ht", [DC * P, S], f16, kind="ExternalInput")
    wqt = nc.dram_tensor("wqt", [D, D], f16, kind="ExternalInput")
    wkt = nc.dram_tensor("wkt", [D, D], f16, kind="ExternalInput")
    wvt = nc.dram_tensor("wvt", [DC * P, HG * DH], f16, kind="ExternalInput")
    mix = nc.dram_tensor("mix", [P, EC * HG], f32, kind="ExternalInput")
    ctx_o = nc.dram_tensor("ctx", [S, HG * DH], f32, kind="ExternalOutput")

    ht_view = ht.rearrange("(c p) s -> p c s", p=P)  # [128, 9, 2048]
    wqt_view = wqt.rearrange("(c p) e -> p c e", p=P)  # [128, 8, 1024]
    wkt_view = wkt.rearrange("(c p) e -> p c e", p=P)
    wvt_view = wvt.rearrange("(c p) n -> p c n", p=P)  # [128, 9, 256]

    with TileContext(nc) as tc:
        with (
            tc.tile_pool(name="const", bufs=1) as cpool,
            tc.tile_pool(name="htp", bufs=2) as htpool,
            tc.tile_pool(name="mqp", bufs=2) as mqpool,
            tc.tile_pool(name="expt", bufs=9) as epool,
            tc.tile_pool(name="ctxt", bufs=5) as ctpool,
            tc.tile_pool(name="outp", bufs=3) as opool,
            tc.tile_pool(name="recp", bufs=4) as rpool,
            tc.tile_pool(name="psm", bufs=4, space="PSUM") as psm,
            tc.tile_pool(name="psc", bufs=4, space="PSUM") as psc,
        ):
            ident = cpool.tile([P, P], f32, tag="ident")
            make_identity(nc, ident)

            # per-chunk DMAs so the first matmuls can start as soon as the
            # chunks they touch have landed (one big DMA delays PE ~15us)
            def load_ht(blk):
                t = htpool.tile([P, DC, NB], f16, tag="htt")
                for d in range(DC):
                    nc.sync.dma_start(
                        t[:, d, :], ht_view[:, d, blk * NB : (blk + 1) * NB]
                    )
                return t

            w_k = cpool.tile([P, EC, D], f16, tag="wk")
            ht0 = htpool.tile([P, DC, NB], f16, tag="htt")
            for d in range(EC):
                nc.sync.dma_start(w_k[:, d, :], wkt_view[:, d, :])
                nc.sync.dma_start(ht0[:, d, :], ht_view[:, d, 0:NB])
            nc.sync.dma_start(ht0[:, EC, :], ht_view[:, EC, 0:NB])
            w_v = cpool.tile([P, DC, HG * DH], f16, tag="wv")
            for d in range(DC):
                nc.sync.dma_start(w_v[:, d, :], wvt_view[:, d, :])

            kt = cpool.tile([P, EC, S], f16, tag="kt")
            # v, padded to 128 columns: [0:64] v-head, 64 ones, [65:128] zero
            vsb = cpool.tile([P, TCH, HG, P], f16, tag="vsb")
            nc.vector.memset(vsb[:], 0.0)
            nc.vector.memset(vsb[:, :, :, DH : DH + 1], 1.0)

            # ---- phase 1: kT (all t) and v (all t) ----
            for tb in range(SBLK):
                htt = ht0 if tb == 0 else load_ht(tb)
                for e in range(EC):
                    ps = psm.tile([P, NB], f32, tag="m")
                    for d in range(EC):
                        nc.tensor.matmul(
                            ps,
                            w_k[:, d, e * P : (e + 1) * P],
                            htt[:, d, :],
                            start=(d == 0),
                            stop=(d == EC - 1),
                        )
                    nc.vector.tensor_copy(kt[:, e, tb * NB : (tb + 1) * NB], ps)
                for ci in range(NB // P):
                    tcc = tb * (NB // P) + ci
                    ps = psm.tile([P, NB], f32, tag="m")
                    psv = ps[:, : HG * DH]
                    for d in range(DC):
                        nc.tensor.matmul(
                            psv,
                            htt[:, d, ci * P : (ci + 1) * P],
                            w_v[:, d, :],
                            start=(d == 0),
                            stop=(d == DC - 1),
                        )
                    for j in range(HG):
                        nc.vector.tensor_copy(
                            vsb[:, tcc, j, 0:DH], psv[:, j * DH : (j + 1) * DH]
                        )

            # weights needed only from phase 2 on: emit after phase 1 so the
            # SP queue prioritizes the phase-1 ht prefetches
            w_q = cpool.tile([P, EC, D], f16, tag="wq")
            for d in range(EC):
                nc.sync.dma_start(w_q[:, d, :], wqt_view[:, d, :])
            mx = cpool.tile([P, EC * HG], f32, tag="mx")
            nc.sync.dma_start(mx[:], mix[:])

            # ---- phase 2: per s-block ----
            # The final ctx matmuls + drain of block i are emitted after the
            # qT matmuls of block i+1, so the PE fills the "waiting for the
            # last exp" pipeline tail with independent projection work.
            def tail_mm(sbi, ctx_ps, last_exp):
                """Last ctx matmuls of block sbi + PSUM->SBUF copies."""
                for j in range(HG):
                    nc.tensor.matmul(
                        ctx_ps[j],
                        vsb[:, TCH - 1, j, :],
                        last_exp[j],
                        start=False,
                        stop=True,
                    )
                ct_tiles = []
                for j in range(HG):
                    ct = ctpool.tile([P, NB], f32, tag="ct", name=f"ct_{sbi}_{j}")
                    nc.vector.tensor_copy(ct[:], ctx_ps[j])
                    ct_tiles.append(ct)
                return ct_tiles

            def make_fin(sbi, ct_tiles):
                """Transpose + normalize + store block sbi. Emitted a couple of
                t-chunks into the next block's scores so the PE transposes don't
                head-of-line-block on the DVE copies."""

                def fin():
                    for sc in range(NB // P):
                        ob = opool.tile([P, HG * DH], f32, tag="ob", name=f"ob_{sbi}_{sc}")
                        for j in range(HG):
                            tp = psm.tile([P, P], f32, tag="m", name=f"tp_{sbi}_{sc}_{j}")
                            nc.tensor.transpose(
                                tp, ct_tiles[j][:, sc * P : (sc + 1) * P], ident
                            )
                            rc = rpool.tile([P, 1], f32, tag="rc", name=f"rc_{sbi}_{sc}_{j}")
                            nc.vector.reciprocal(rc, tp[:, DH : DH + 1])
                            nc.vector.tensor_tensor(
                                ob[:, j * DH : (j + 1) * DH],
                                tp[:, 0:DH],
                                rc[:, 0, None].to_broadcast([P, DH]),
                                mult,
                            )
                        row0 = sbi * NB + sc * P
                        # gpsimd (SWDGE) keeps output stores off the SP queue,
                        # which is busy prefetching ht slices
                        nc.gpsimd.dma_start(ctx_o[row0 : row0 + P, :], ob[:])

                return fin

            pending_mm = None
            for sbi in range(SBLK):
                htt = load_ht(sbi)
                mq = mqpool.tile([P, EC, HG, NB], f16, tag="mq")
                for e in range(EC):
                    ps = psm.tile([P, NB], f32, tag="m")
                    for d in range(EC):
                        nc.tensor.matmul(
                            ps,
                            w_q[:, d, e * P : (e + 1) * P],
                            htt[:, d, :],
                            start=(d == 0),
                            stop=(d == EC - 1),
                        )
                    for j in range(HG):
                        # split the 4 per-head multiplies across DVE and ACT:
                        # either engine alone is slower than the PE's 8 matmuls
                        # per e-chunk and would stall the projection pipeline
                        if j % 2 == 0:
                            nc.vector.tensor_tensor(
                                mq[:, e, j, :],
                                ps,
                                mx[:, e * HG + j, None].to_broadcast([P, NB]),
                                mult,
                            )
                        else:
                            nc.scalar.activation(
                                mq[:, e, j, :],
                                ps,
                                Copy,
                                scale=mx[:, e * HG + j, None],
                            )

                pending_fin = None
                if pending_mm is not None:
                    prev_sbi, prev_ctx_ps, prev_last_exp = pending_mm
                    ct_tiles = tail_mm(prev_sbi, prev_ctx_ps, prev_last_exp)
                    pending_fin = make_fin(prev_sbi, ct_tiles)

                ctx_ps = [
                    psc.tile([P, NB], f32, tag="c", name=f"ctxps_{sbi}_{j}")
                    for j in range(HG)
                ]
                prev_exp = [None] * HG
                for tci in range(TCH):
                    cur_exp = []
                    for j in range(HG):
                        sp = psm.tile([P, NB], f32, tag="m")
                        for e in range(EC):
                            nc.tensor.matmul(
                                sp,
                                kt[:, e, tci * P : (tci + 1) * P],
                                mq[:, e, j, :],
                                start=(e == 0),
                                stop=(e == EC - 1),
                            )
                        et = epool.tile([P, NB], f16, tag="et")
                        nc.scalar.activation(et, sp, Exp, scale=SCALE)
                        cur_exp.append(et)
                    if tci > 0:
                        for j in range(HG):
                            nc.tensor.matmul(
                                ctx_ps[j],
                                vsb[:, tci - 1, j, :],
                                prev_exp[j],
                                start=(tci - 1 == 0),
                                stop=False,
                            )
                    prev_exp = cur_exp
                    if tci == 1 and pending_fin is not None:
                        pending_fin()
                        pending_fin = None

                pending_mm = (sbi, ctx_ps, prev_exp)

            ct_tiles = tail_mm(*pending_mm)
            make_fin(pending_mm[0], ct_tiles)()

    nc.compile()
    return nc


def make_in_maps(hidden_states, Wq, Wk, Wv, bv, mixing):
    """Host-side sharding: build per-core input dicts."""
    hidden_states = np.asarray(hidden_states, dtype=np.float32)
    Wq = np.asarray(Wq, dtype=np.float32)
    Wk = np.asarray(Wk, dtype=np.float32)
    Wv = np.asarray(Wv, dtype=np.float32)
    bv = np.asarray(bv, dtype=np.float32)
    mixing = np.asarray(mixing, dtype=np.float32)

    bf = np.float16
    wqt = np.ascontiguousarray(Wq.T).astype(bf)  # [d, e]
    wkt = np.ascontiguousarray(Wk.T).astype(bf)

    ht_by_b = []
    for b in range(B):
        ht = np.zeros((DC * P, S), dtype=bf)
        ht[:D] = hidden_states[b].T.astype(bf)
        ht[D] = 1.0
        ht_by_b.append(ht)

    wvt_by_g = []
    mix_by_g = []
    wvT = Wv.T  # [d, dv]
    for g in range(HG):
        wvt = np.zeros((DC * P, HG * DH), dtype=bf)
        wvt[:D] = wvT[:, g * HG * DH : (g + 1) * HG * DH].astype(bf)
        wvt[D] = bv[g * HG * DH : (g + 1) * HG * DH].astype(bf)
        wvt_by_g.append(wvt)
        mrows = mixing[g * HG : (g + 1) * HG]  # [4, 1024]
        # mix[p, e*HG + j] = mixing[4g+j, e*128+p]
        m = np.ascontiguousarray(
            mrows.reshape(HG, EC, P).transpose(2, 1, 0).reshape(P, EC * HG)
        ).astype(np.float32)
        mix_by_g.append(m)

    in_maps = []
    for c in range(N_CORES):
        b, g = divmod(c, HG)
        in_maps.append(
            {
                "ht": ht_by_b[b],
                "wqt": wqt,
                "wkt": wkt,
                "wvt": wvt_by_g[g],
                "mix": mix_by_g[g],
            }
        )
    return in_maps


def assemble_output(results):
    """results: list of per-core dicts with 'ctx' [S, 256] f32."""
    out = np.empty((B, S, DV), dtype=np.float32)
    for c in range(N_CORES):
        b, g = divmod(c, HG)
        out[b, :, g * HG * DH : (g + 1) * HG * DH] = results[c]["ctx"]
    return out


def kernel(hidden_states, Wq, Wk, Wv, bv, mixing):
    from concourse.bass_utils import run_bass_kernel_spmd

    if "nc" not in _CACHE:
        _CACHE["nc"] = build_program()
    nc = _CACHE["nc"]
    in_maps = make_in_maps(hidden_states, Wq, Wk, Wv, bv, mixing)
    res = run_bass_kernel_spmd(nc, in_maps, list(range(N_CORES)))
    return assemble_output(res.results)


# revision 21
# speedup vs baseline: 1.2027x; 1.2027x over previous
"""Collaborative attention (nn_CollaborativeAttention) on 8 Trainium2 NeuronCores.

Reference math (B=2, S=2048, D=1024, H=16 heads, head mixing over full DKQ=1024):
    q = h @ Wq.T ; k = h @ Wk.T ; v = h @ Wv.T + bv
    scores[b,h,s,t] = sum_e q[b,s,e] * mixing[h,e] * k[b,t,e] / sqrt(64)
    probs = softmax_t(scores) ; ctx[b,s,:] = concat_h(probs @ v_head)

Sharding: core c handles batch b = c//4 and head group g = c%4 (4 heads each).
No cross-core communication; host slices inputs / concatenates outputs.

Device dataflow (per core, all matmuls f16 with fp32 PSUM accumulation):
    hT_aug [1152,2048]  host-transposed hidden (+ ones row for the v bias)
    kT[e,t]  = WkT.T @ hT          (stays transposed: e on partitions)
    v[t,dv]  = hT.T @ WvT_aug      (natural; padded to 128 cols: col 64 = ones
                                    row -> softmax denominator, cols 65.. = 0)
    per s-block of 512 queries:
      mq[e,s]   = (WqT.T @ hT_s) * mixing[h,e]        (per head)
      scoresT   = kT_chunk.T @ mq  -> exp(0.125*x) -> expT[t,s] (f16)
      ctxT_aug  = sum_t v_chunk.T @ expT   (PSUM accum over 16 t-chunks;
                 rows 0..63 = unnormalized ctxT, row 64 = denominator)
      PE-transpose ctxT_aug 128x128 blocks -> ctx[s,dv] ; multiply by
      reciprocal(denominator) ; DMA out.
"""

import math

import numpy as np

B, S, D = 2, 2048, 1024
H, DV = 16, 1024
N_CORES = 8
HG = 4  # heads per core
DH = 64  # head dim
P = 128
EC = 8  # e-chunks (1024/128)
DC = 9  # d-chunks incl. bias row (1152/128)
NB = 512  # s-block width
SBLK = 4  # number of s blocks
TCH = 16  # t-chunks of 128
SCALE = 1.0 / math.sqrt(D / H)  # 0.125

_CACHE: dict = {}


def build_program():
    """Build the (SPMD, per-core) Bass program. Returns nc (compiled Bacc —
    Bacc.compile() runs the wait-legalization passes walrus needs: at most one
    semaphore wait per instruction, excess split into EventSemaphore chains)."""
    import concourse.bass as bass
    import concourse.mybir as mybir
    from concourse import bacc
    from concourse.tile import TileContext
    from concourse.masks import make_identity

    f32 = mybir.dt.float32
    f16 = mybir.dt.float16
    mult = mybir.AluOpType.mult
    Exp = mybir.ActivationFunctionType.Exp
    Copy = mybir.ActivationFunctionType.Copy

    nc = bacc.Bacc("TRN2", target_bir_lowering=False, debug=True)
    ht = nc.dram_tensor("ht", [DC * P, S], f16, kind="ExternalInput")
    wqt = nc.dram_tensor("wqt", [D, D], f16, kind="ExternalInput")
    wkt = nc.dram_tensor("wkt", [D, D], f16, kind="ExternalInput")
    wvt = nc.dram_tensor("wvt", [DC * P, HG * DH], f16, kind="ExternalInput")
    mix = nc.dram_tensor("mix", [P, EC * HG], f32, kind="ExternalInput")
    ctx_o = nc.dram_tensor("ctx", [S, HG * DH], f32, kind="ExternalOutput")

    ht_view = ht.rearrange("(c p) s -> p c s", p=P)  # [128, 9, 2048]
    wqt_view = wqt.rearrange("(c p) e -> p c e", p=P)  # [128, 8, 1024]
    wkt_view = wkt.rearrange("(c p) e -> p c e", p=P)
    wvt_view = wvt.rearrange("(c p) n -> p c n", p=P)  # [128, 9, 256]

    with TileContext(nc) as tc:
        with (
            tc.tile_pool(name="const", bufs=1) as cpool,
            tc.tile_pool(name="htp", bufs=2) as htpool,
            tc.tile_pool(name="mqp", bufs=2) as mqpool,
            tc.tile_pool(name="expt", bufs=9) as epool,
            tc.tile_pool(name="ctxt", bufs=5) as ctpool,
            tc.tile_pool(name="outp", bufs=3) as opool,
            tc.tile_pool(name="recp", bufs=4) as rpool,
            tc.tile_pool(name="psm", bufs=4, space="PSUM") as psm,
            tc.tile_pool(name="psc", bufs=4, space="PSUM") as psc,
        ):
            ident = cpool.tile([P, P], f32, tag="ident")
            make_identity(nc, ident)

            # per-chunk DMAs so the first matmuls can start as soon as the
            # chunks they touch have landed (one big DMA delays PE ~15us)
            def load_ht(blk):
                t = htpool.tile([P, DC, NB], f16, tag="htt")
                for d in range(DC):
                    # alternate issue queues: SP dma_start issue is ~0.5us, so a
                    # 9-chunk load serializes ~4.5us on one sequencer
                    eng = nc.sync if d % 2 == 0 else nc.gpsimd
                    eng.dma_start(t[:, d, :], ht_view[:, d, blk * NB : (blk + 1) * NB])
                return t

            w_k = cpool.tile([P, EC, D], f16, tag="wk")
            ht0 = htpool.tile([P, DC, NB], f16, tag="htt")
            for d in range(EC):
                nc.sync.dma_start(w_k[:, d, :], wkt_view[:, d, :])
                nc.gpsimd.dma_start(ht0[:, d, :], ht_view[:, d, 0:NB])
            nc.gpsimd.dma_start(ht0[:, EC, :], ht_view[:, EC, 0:NB])
            w_v = cpool.tile([P, DC, HG * DH], f16, tag="wv")
            for d in range(DC):
                nc.sync.dma_start(w_v[:, d, :], wvt_view[:, d, :])

            kt = cpool.tile([P, EC, S], f16, tag="kt")
            # v, padded to 128 columns: [0:64] v-head, 64 ones, [65:128] zero
            vsb = cpool.tile([P, TCH, HG, P], f16, tag="vsb")
            nc.vector.memset(vsb[:], 0.0)
            nc.vector.memset(vsb[:, :, :, DH : DH + 1], 1.0)

            # ---- phase 1: kT (all t) and v (all t) ----
            for tb in range(SBLK):
                htt = ht0 if tb == 0 else load_ht(tb)
                for e in range(EC):
                    ps = psm.tile([P, NB], f32, tag="m")
                    for d in range(EC):
                        nc.tensor.matmul(
                            ps,
                            w_k[:, d, e * P : (e + 1) * P],
                            htt[:, d, :],
                            start=(d == 0),
                            stop=(d == EC - 1),
                        )
                    nc.vector.tensor_copy(kt[:, e, tb * NB : (tb + 1) * NB], ps)
                for ci in range(NB // P):
                    tcc = tb * (NB // P) + ci
                    ps = psm.tile([P, NB], f32, tag="m")
                    psv = ps[:, : HG * DH]
                    for d in range(DC):
                        nc.tensor.matmul(
                            psv,
                            htt[:, d, ci * P : (ci + 1) * P],
                            w_v[:, d, :],
                            start=(d == 0),
                            stop=(d == DC - 1),
                        )
                    for j in range(HG):
                        nc.vector.tensor_copy(
                            vsb[:, tcc, j, 0:DH], psv[:, j * DH : (j + 1) * DH]
                        )

            # weights needed only from phase 2 on: emit after phase 1 so the
            # SP queue prioritizes the phase-1 ht prefetches
            w_q = cpool.tile([P, EC, D], f16, tag="wq")
            for d in range(EC):
                nc.sync.dma_start(w_q[:, d, :], wqt_view[:, d, :])
            mx = cpool.tile([P, EC * HG], f32, tag="mx")
            nc.sync.dma_start(mx[:], mix[:])

            # ---- phase 2: per s-block ----
            # The final ctx matmuls + drain of block i are emitted after the
            # qT matmuls of block i+1, so the PE fills the "waiting for the
            # last exp" pipeline tail with independent projection work.
            def tail_mm(sbi, ctx_ps, last_exp):
                """Last ctx matmuls of block sbi + PSUM->SBUF copies."""
                for j in range(HG):
                    nc.tensor.matmul(
                        ctx_ps[j],
                        vsb[:, TCH - 1, j, :],
                        last_exp[j],
                        start=False,
                        stop=True,
                    )
                ct_tiles = []
                for j in range(HG):
                    ct = ctpool.tile([P, NB], f32, tag="ct", name=f"ct_{sbi}_{j}")
                    nc.vector.tensor_copy(ct[:], ctx_ps[j])
                    ct_tiles.append(ct)
                return ct_tiles

            def make_fin(sbi, ct_tiles):
                """Transpose + normalize + store block sbi. Emitted a couple of
                t-chunks into the next block's scores so the PE transposes don't
                head-of-line-block on the DVE copies."""

                def fin():
                    for sc in range(NB // P):
                        ob = opool.tile([P, HG * DH], f32, tag="ob", name=f"ob_{sbi}_{sc}")
                        for j in range(HG):
                            tp = psm.tile([P, P], f32, tag="m", name=f"tp_{sbi}_{sc}_{j}")
                            nc.tensor.transpose(
                                tp, ct_tiles[j][:, sc * P : (sc + 1) * P], ident
                            )
                            rc = rpool.tile([P, 1], f32, tag="rc", name=f"rc_{sbi}_{sc}_{j}")
                            nc.vector.reciprocal(rc, tp[:, DH : DH + 1])
                            nc.vector.tensor_tensor(
                                ob[:, j * DH : (j + 1) * DH],
                                tp[:, 0:DH],
                                rc[:, 0, None].to_broadcast([P, DH]),
                                mult,
                            )
                        row0 = sbi * NB + sc * P
                        # gpsimd (SWDGE) keeps output stores off the SP queue,
                        # which is busy prefetching ht slices
                        nc.gpsimd.dma_start(ctx_o[row0 : row0 + P, :], ob[:])

                return fin

            pending_mm = None
            for sbi in range(SBLK):
                htt = load_ht(sbi)
                mq = mqpool.tile([P, EC, HG, NB], f16, tag="mq")
                for e in range(EC):
                    ps = psm.tile([P, NB], f32, tag="m")
                    for d in range(EC):
                        nc.tensor.matmul(
                            ps,
                            w_q[:, d, e * P : (e + 1) * P],
                            htt[:, d, :],
                            start=(d == 0),
                            stop=(d == EC - 1),
                        )
                    for j in range(HG):
                        # split the 4 per-head multiplies across DVE and ACT:
                        # either engine alone is slower than the PE's 8 matmuls
                        # per e-chunk and would stall the projection pipeline
                        if j % 2 == 0:
                            nc.vector.tensor_tensor(
                                mq[:, e, j, :],
                                ps,
                                mx[:, e * HG + j, None].to_broadcast([P, NB]),
                                mult,
                            )
                        else:
                            nc.scalar.activation(
                                mq[:, e, j, :],
                                ps,
                                Copy,
                                scale=mx[:, e * HG + j, None],
                            )

                pending_fin = None
                if pending_mm is not None:
                    prev_sbi, prev_ctx_ps, prev_last_exp = pending_mm
                    ct_tiles = tail_mm(prev_sbi, prev_ctx_ps, prev_last_exp)
                    pending_fin = make_fin(prev_sbi, ct_tiles)

                ctx_ps = [
                    psc.tile([P, NB], f32, tag="c", name=f"ctxps_{sbi}_{j}")
                    for j in range(HG)
                ]
                prev_exp = [None] * HG
                for tci in range(TCH):
                    cur_exp = []
                    for j in range(HG):
                        sp = psm.tile([P, NB], f32, tag="m")
                        for e in range(EC):
                            nc.tensor.matmul(
                                sp,
                                kt[:, e, tci * P : (tci + 1) * P],
                                mq[:, e, j, :],
                                start=(e == 0),
                                stop=(e == EC - 1),
                            )
                        et = epool.tile([P, NB], f16, tag="et")
                        nc.scalar.activation(et, sp, Exp, scale=SCALE)
                        cur_exp.append(et)
                    if tci > 0:
                        for j in range(HG):
                            nc.tensor.matmul(
                                ctx_ps[j],
                                vsb[:, tci - 1, j, :],
                                prev_exp[j],
                                start=(tci - 1 == 0),
                                stop=False,
                            )
                    prev_exp = cur_exp
                    if tci == 1 and pending_fin is not None:
                        pending_fin()
                        pending_fin = None

                pending_mm = (sbi, ctx_ps, prev_exp)

            ct_tiles = tail_mm(*pending_mm)
            make_fin(pending_mm[0], ct_tiles)()

    nc.compile()
    return nc


def make_in_maps(hidden_states, Wq, Wk, Wv, bv, mixing):
    """Host-side sharding: build per-core input dicts."""
    hidden_states = np.asarray(hidden_states, dtype=np.float32)
    Wq = np.asarray(Wq, dtype=np.float32)
    Wk = np.asarray(Wk, dtype=np.float32)
    Wv = np.asarray(Wv, dtype=np.float32)
    bv = np.asarray(bv, dtype=np.float32)
    mixing = np.asarray(mixing, dtype=np.float32)

    bf = np.float16
    wqt = np.ascontiguousarray(Wq.T).astype(bf)  # [d, e]
    wkt = np.ascontiguousarray(Wk.T).astype(bf)

    ht_by_b = []
    for b in range(B):
        ht = np.zeros((DC * P, S), dtype=bf)
        ht[:D] = hidden_states[b].T.astype(bf)
        ht[D] = 1.0
        ht_by_b.append(ht)

    wvt_by_g = []
    mix_by_g = []
    wvT = Wv.T  # [d, dv]
    for g in range(HG):
        wvt = np.zeros((DC * P, HG * DH), dtype=bf)
        wvt[:D] = wvT[:, g * HG * DH : (g + 1) * HG * DH].astype(bf)
        wvt[D] = bv[g * HG * DH : (g + 1) * HG * DH].astype(bf)
        wvt_by_g.append(wvt)
        mrows = mixing[g * HG : (g + 1) * HG]  # [4, 1024]
        # mix[p, e*HG + j] = mixing[4g+j, e*128+p]
        m = np.ascontiguousarray(
            mrows.reshape(HG, EC, P).transpose(2, 1, 0).reshape(P, EC * HG)
        ).astype(np.float32)
        mix_by_g.append(m)

    in_maps = []
    for c in range(N_CORES):
        b, g = divmod(c, HG)
        in_maps.append(
            {
                "ht": ht_by_b[b],
                "wqt": wqt,
                "wkt": wkt,
                "wvt": wvt_by_g[g],
                "mix": mix_by_g[g],
            }
        )
    return in_maps


def assemble_output(results):
    """results: list of per-core dicts with 'ctx' [S, 256] f32."""
    out = np.empty((B, S, DV), dtype=np.float32)
    for c in range(N_CORES):
        b, g = divmod(c, HG)
        out[b, :, g * HG * DH : (g + 1) * HG * DH] = results[c]["ctx"]
    return out


def _get_runner():
    """Build (once) a jitted shard_map over the 8 cores running the compiled
    Bass program via the bass_exec custom call. Mirrors
    concourse.bass2jax.run_bass_via_pjrt, but caches the jitted callable so
    repeat kernel() calls skip re-lowering."""
    if "runner" in _CACHE:
        return _CACHE["runner"]

    import jax
    import concourse.mybir as mybir
    from jax.sharding import Mesh, PartitionSpec
    from jax.experimental.shard_map import shard_map
    from concourse import bass2jax
    from concourse.bass2jax import _bass_exec_p, partition_id_tensor

    bass2jax.install_neuronx_cc_hook()
    nc = _CACHE.setdefault("nc", build_program())

    part_name = nc.partition_id_tensor.name if nc.partition_id_tensor else None
    dbg_name = nc.dbg_addr.name if nc.dbg_addr is not None else None
    in_names, out_names, out_avals, zero_outs = [], [], [], []
    for alloc in nc.m.functions[0].allocations:
        if not isinstance(alloc, mybir.MemoryLocationSet):
            continue
        name = alloc.memorylocations[0].name
        if alloc.kind == "ExternalInput":
            if name != part_name:
                in_names.append(name)
        elif alloc.kind == "ExternalOutput":
            out_names.append(name)
            shape = tuple(alloc.tensor_shape)
            dtype = mybir.dt.np(alloc.dtype)
            out_avals.append(jax.core.ShapedArray(shape, dtype))
            zero_outs.append(np.zeros(shape, dtype))
    n_params = len(in_names)
    all_names = in_names + out_names + ([part_name] if part_name else [])

    def _body(*args):
        operands = list(args)
        if part_name is not None:
            operands.append(partition_id_tensor())
        outs = _bass_exec_p.bind(
            *operands,
            out_avals=tuple(out_avals),
            in_names=tuple(all_names),
            out_names=tuple(out_names),
            lowering_input_output_aliases=(),
            sim_require_finite=True,
            sim_require_nnan=True,
            nc=nc,
        )
        return tuple(outs)

    devices = jax.devices()[:N_CORES]
    mesh = Mesh(np.asarray(devices), ("core",))
    spec = PartitionSpec("core")
    sharded = jax.jit(
        shard_map(
            _body,
            mesh=mesh,
            in_specs=(spec,) * (n_params + len(out_names)),
            out_specs=(spec,) * len(out_names),
            check_rep=False,
        ),
        keep_unused=True,
    )
    concat_zero = [
        np.zeros((N_CORES * z.shape[0], *z.shape[1:]), z.dtype) for z in zero_outs
    ]

    def run(in_maps):
        def core_input(c, name):
            if name == dbg_name:
                return np.zeros((1, 2), np.uint32)
            return in_maps[c][name]

        concat_in = [
            np.concatenate([core_input(c, name) for c in range(N_CORES)], axis=0)
            for name in in_names
        ]
        out_arrs = sharded(*concat_in, *concat_zero)
        return [
            {
                name: np.asarray(out_arrs[i]).reshape(
                    N_CORES, *out_avals[i].shape
                )[c]
                for i, name in enumerate(out_names)
            }
            for c in range(N_CORES)
        ]

    _CACHE["runner"] = run
    return run


def kernel(hidden_states, Wq, Wk, Wv, bv, mixing):
    run = _get_runner()
    in_maps = make_in_maps(hidden_states, Wq, Wk, Wv, bv, mixing)
    return assemble_output(run(in_maps))
